# revision 1
# baseline (speedup 1.0000x reference)
"""SoftRas-style soft rasterizer on 8 Trainium2 NeuronCores.

Strategy:
- All per-(face,pixel) affine quantities (barycentric w0/w1, edge projections
  u/l2, squared vertex distances) are produced by TensorE matmuls against the
  pixel basis [1, px, py, px^2+py^2] (K=4).
- The nonlinear chain (clip/sqrt/sigmoid/reciprocal/exp/log) runs on
  VectorE/ScalarE with faces on partitions (128/chunk), pixels on the free dim.
- Per-pixel max over faces (softmax shift) via PE transposes + free-dim max.
- Face-direction reductions (rgb accumulation, dsum, sum log(1-p)) via PE
  matmuls against textures / ones.
- Host (numpy): per-face coefficient prep, per-tile face culling, per-core
  load balancing (every core gets an identical chunk-count pattern so one
  SPMD program serves all 8 cores), final divide + alpha exponentiation.
"""
import sys
sys.path.insert(0, '/opt/trn_rl_repo')
import numpy as np
import ml_dtypes
from contextlib import ExitStack

import concourse.bass as bass
import concourse.bacc as bacc
import concourse.tile as tile
import concourse.mybir as mybir
from concourse.bass_utils import run_bass_kernel_spmd
from concourse.masks import make_identity

TRACE = False
LAST_RESULT = None

F_TOT = 512
H = W = 256
NCORES = 8
TP = 512              # pixels per tile position (2 image rows)
NPOS = (H * W) // (NCORES * TP)   # 16 tile positions per core
SIGMA = 1e-2
GAMMA = 1e-3
EPS = 1e-3
NEAR, FAR = 1.0, 100.0
FP = mybir.dt.float32
F32R = mybir.dt.float32r
BF = mybir.dt.bfloat16
AL = mybir.AluOpType
AF = mybir.ActivationFunctionType


def _host_prep(face_vertices):
    """Per-face coefficients in basis [1, px, py, r2], fp64 -> fp32."""
    fv = np.asarray(face_vertices, np.float64)[0]          # [F,3,3]
    F = fv.shape[0]
    x = fv[:, :, 0]; y = fv[:, :, 1]; z = fv[:, :, 2]
    x0, x1, x2 = x[:, 0], x[:, 1], x[:, 2]
    y0, y1, y2 = y[:, 0], y[:, 1], y[:, 2]

    den = (y1 - y2) * (x0 - x2) + (x2 - x1) * (y0 - y2)
    den = np.where(np.abs(den) < 1e-10, 1e-10, den)
    W0c = np.stack([(-(y1 - y2) * x2 - (x2 - x1) * y2) / den,
                    (y1 - y2) / den, (x2 - x1) / den, np.zeros(F)], -1)
    W1c = np.stack([(-(y2 - y0) * x2 - (x0 - x2) * y2) / den,
                    (y2 - y0) / den, (x0 - x2) / den, np.zeros(F)], -1)

    anchors = [(x0, y0), (x1, y1), (x2, y2)]
    pairs = [(0, 1), (1, 2), (2, 0)]
    # per edge: U = ((p-a).d)/|d| (along-line coord), LD = cross(p-a, d)/|d|
    # (signed line distance). d2seg = LD^2 + max(|U - L/2| - L/2, 0)^2.
    UT = np.zeros((3, F, 4)); S2 = np.zeros((3, F, 4)); HL = np.zeros((3, F))
    for e, (ia, ib) in enumerate(pairs):
        ax, ay = anchors[ia]; bx, by = anchors[ib]
        dx, dy = bx - ax, by - ay
        L = np.sqrt(np.maximum(dx * dx + dy * dy, 1e-12))
        iL = 1.0 / L
        UT[e, :, 0] = (-ax * dx - ay * dy) * iL - L / 2.0   # y = U - L/2
        UT[e, :, 1] = dx * iL
        UT[e, :, 2] = dy * iL
        S2[e, :, 0] = (ay * dx - ax * dy) * iL
        S2[e, :, 1] = dy * iL
        S2[e, :, 2] = -dx * iL
        HL[e] = L / 2.0
    iz = 1.0 / z
    zmin = z.min(1); zmax = z.max(1)
    assert z.min() > NEAR + 0.05 and z.max() < FAR - 0.05, \
        "kernel fast path assumes all vertex depths strictly inside (NEAR,FAR)"
    return dict(W0c=W0c, W1c=W1c, UT=UT, S2=S2, HL=HL, iz=iz,
                ymin=y.min(1), ymax=y.max(1), xmin=x.min(1), xmax=x.max(1),
                zmin=zmin, zmax=zmax)


def _cull_and_balance(prep):
    """Per tile (4 rows x 128 px), the kept-face list; balanced so all cores
    share one chunk-count pattern. Returns (pattern, assign) where
    assign[core][pos] = (tile_index, face_index_array padded with -1)."""
    nyb = H // 4
    pixc = ((np.arange(H) + 0.5) / H) * 2.0 - 1.0
    tiles = []
    for yb in range(nyb):
        for xb in range(2):
            tiles.append((pixc[4 * yb], pixc[4 * yb + 3],
                          pixc[128 * xb], pixc[128 * xb + 127]))
    tiles = np.array(tiles)                                # [nb, 4]
    nb = len(tiles)
    ygap = np.maximum(0.0, np.maximum(
        prep['ymin'][None, :] - tiles[:, 1:2],
        tiles[:, 0:1] - prep['ymax'][None, :]))
    xgap = np.maximum(0.0, np.maximum(
        prep['xmin'][None, :] - tiles[:, 3:4],
        tiles[:, 2:3] - prep['xmax'][None, :]))
    gap = np.sqrt(xgap ** 2 + ygap ** 2)
    znUB = (FAR - prep['zmin']) / (FAR - NEAR)
    znLB = (FAR - prep['zmax']) / (FAR - NEAR)
    D = znLB.max()
    MH = znUB.max()
    # every pixel's true zmax lies in [D, MH]; if that window is narrow a
    # single global softmax shift MH is exact (no over/underflow possible)
    assert MH - D <= 0.07, "global-shift fast path needs a narrow zmax window"
    keep = (gap < 0.17) | ((gap / SIGMA) + (D - znUB) / GAMMA < 87.0)  # [nb,F]

    counts = np.maximum(1, np.ceil(keep.sum(1) / 128).astype(int))
    order = np.argsort(-counts, kind='stable')             # bands, desc count
    pattern = [int(counts[order[p * NCORES]]) for p in range(NPOS)]
    assign = [[None] * NPOS for _ in range(NCORES)]
    for p in range(NPOS):
        for c in range(NCORES):
            b = int(order[p * NCORES + c])
            faces = np.nonzero(keep[b])[0]
            pad = pattern[p] * 128 - len(faces)
            assert pad >= 0
            faces = np.concatenate([faces, -np.ones(pad, np.int64)])
            assign[c][p] = (b, faces)
    return pattern, assign, float(max(MH, EPS))


# 3-way bf16 split: x = h + m + l with each part bf16-exact. Products of
# bf16-exact values are exact in the PE's f32r mode, so a 6-combo expansion
# (dropping <1e-7 cross terms) gives fp32-class precision at full PE rate.
COMBOS = [(0, 0), (0, 1), (1, 0), (0, 2), (1, 1), (2, 0)]
NK = 4 * len(COMBOS)


def _split3(a):
    a = np.asarray(a, np.float64)
    h = a.astype(ml_dtypes.bfloat16).astype(np.float64)
    r = a - h
    m = r.astype(ml_dtypes.bfloat16).astype(np.float64)
    l = (r - m).astype(ml_dtypes.bfloat16).astype(np.float64)
    return [h, m, l]


def _face_arrays(prep, textures, faces):
    """Pack per-chunk coefficient/texture/scalar arrays for one chunk of 128
    face slots (index -1 = inert dummy)."""
    f = np.asarray(faces)
    dummy = f < 0
    fi = np.where(dummy, 0, f)

    def D(a):  # zero out dummies
        a = np.asarray(a, np.float64).copy()
        a[dummy] = 0.0
        return a

    # quantity order: U01,LD01,U12,LD12,U20,LD20,W0,W1,W2 -> coef[4, 9, 128]
    coef = np.zeros((4, 9, 128))
    for e in range(3):
        coef[:, 2 * e, :] = D(prep['UT'][e][fi]).T
        coef[:, 2 * e + 1, :] = D(prep['S2'][e][fi]).T
    coef[:, 6, :] = D(prep['W0c'][fi]).T
    coef[:, 7, :] = D(prep['W1c'][fi]).T
    # dummies: W0=W1=-1 (outside, wc2=1), LD=10 (dist 10 -> prob 0),
    # iz=0.011 -> zp~90.9 -> zn~0.092 (never the argmax), halfL=0.5
    coef[0, 1, dummy] = 10.0
    coef[0, 3, dummy] = 10.0
    coef[0, 5, dummy] = 10.0
    coef[0, 6, dummy] = -1.0
    coef[0, 7, dummy] = -1.0
    coef[:, 8, :] = -coef[:, 6, :] - coef[:, 7, :]
    coef[0, 8, :] += 1.0                                   # w2 = 1 - w0 - w1
    cs = _split3(coef)
    coefk = np.zeros((NK, 9, 128), np.float32)
    for t, (ci, bi) in enumerate(COMBOS):
        coefk[4 * t:4 * t + 4] = cs[ci].astype(np.float32)

    tex = np.asarray(textures, np.float64)[0][fi]          # [128,3,3] (k,c)
    tex[dummy] = 0.0

    scal = np.zeros((128, 9))
    izf = prep['iz'][fi]
    izf[dummy] = 0.011
    scal[:, 0:3] = izf
    hlf = prep['HL'][:, fi].T
    hlf[dummy] = 0.5
    scal[:, 3:6] = hlf
    scal[:, 6:9] = -hlf
    return coefk, tex, scal


def _build_program(pattern, mhat):
    """One SPMD Bass program; chunk counts per position given by pattern."""
    totc = sum(pattern)
    kmax = max(pattern)
    nc = bacc.Bacc("TRN2", target_bir_lowering=False, debug=False,
                   num_devices=NCORES)
    d_coef = nc.dram_tensor("coef", [totc, 24, 9 * 128], F32R, kind="ExternalInput")
    d_basis = nc.dram_tensor("basis", [NPOS, 24, TP], F32R, kind="ExternalInput")
    d_tex = nc.dram_tensor("tex", [128, totc * 9], FP, kind="ExternalInput")
    d_scal = nc.dram_tensor("scal", [128, totc * 9], FP, kind="ExternalInput")
    d_out = nc.dram_tensor("out6", [5, NPOS * TP], FP, kind="ExternalOutput")

    with ExitStack() as ctx:
        tc = ctx.enter_context(tile.TileContext(nc))
        const = ctx.enter_context(tc.tile_pool(name="const", bufs=1))
        stage = ctx.enter_context(tc.tile_pool(name="stage", bufs=3))
        basp = ctx.enter_context(tc.tile_pool(name="basp", bufs=3))
        work = ctx.enter_context(tc.tile_pool(name="work", bufs=2))
        store = ctx.enter_context(tc.tile_pool(name="store", bufs=3))
        zm = ctx.enter_context(tc.tile_pool(name="zm", bufs=3))
        qp = ctx.enter_context(tc.tile_pool(name="qp", bufs=6, space="PSUM"))
        accp = ctx.enter_context(tc.tile_pool(name="accp", bufs=2, space="PSUM"))

        onesc = const.tile([128, 1], BF)
        nc.vector.memset(onesc, 1.0)
        onesf = const.tile([128, 1], FP)
        nc.vector.memset(onesf, 1.0)
        b_sqrt = const.tile([128, 1], FP)
        nc.vector.memset(b_sqrt, 1e-12)
        b_ln = const.tile([128, 1], FP)
        nc.vector.memset(b_ln, 1e-30)
        b_exp = const.tile([128, 1], FP)
        nc.vector.memset(b_exp, -mhat / GAMMA)
        tex_sb = const.tile([128, totc * 9], FP)
        nc.sync.dma_start(out=tex_sb, in_=d_tex[:, :])
        scal_sb = const.tile([128, totc * 9], FP)
        nc.sync.dma_start(out=scal_sb, in_=d_scal[:, :])

        jj = 0
        base = [0]
        for p in pattern:
            base.append(base[-1] + p)
        for pp in range(0, NPOS, 2):
            pair = [pp, pp + 1]
            st8 = {}
            for pos in pair:
                K = pattern[pos]
                bas = basp.tile([24, TP], F32R, tag="bas")
                nc.sync.dma_start(out=bas, in_=d_basis[pos, :, :])

                zn_st = store.tile([128, kmax, TP], FP, tag="zn_st")
                pv_st = store.tile([128, kmax, TP], FP, tag="pv_st")
                wn_st = [store.tile([128, kmax, TP], FP, tag=f"wn{k}_st",
                                     name=f"wn{k}_st") for k in range(3)]
                d2_st = store.tile([128, kmax, TP], FP, tag="d2_st")
                sb_st = store.tile([128, kmax, TP], mybir.dt.uint32,
                                   tag="sb_st")
                acc = accp.tile([65, TP], FP, tag="acc")
                st8[pos] = (zn_st, pv_st, wn_st, d2_st, sb_st, acc)

                # -- phase 1a: per chunk, everything except sqrt/sig/ln/exp --
                for j in range(K):
                    cj = base[pos] + j
                    st = stage.tile([24, 9 * 128], F32R, tag="st")
                    nc.sync.dma_start(out=st, in_=d_coef[cj, :, :])
                    q = [qp.tile([128, TP], FP, tag="q", name=f"q{qi}")
                         for qi in range(9)]
                    for qi in range(9):
                        nc.tensor.matmul(q[qi],
                                         st[:, qi * 128:(qi + 1) * 128],
                                         bas, start=True, stop=True)
                    sc = lambda i: scal_sb[:, cj * 9 + i: cj * 9 + i + 1]

                    # d2_e = LD^2 + relu(|U - L/2| - L/2)^2 (U-L/2 from PE)
                    ru = [work.tile([128, TP], FP, tag=f"ru{e}",
                                    name=f"ru{e}") for e in range(3)]
                    tt = [work.tile([128, TP], FP, tag=f"t{e}",
                                    name=f"t{e}") for e in range(3)]
                    for e in range(3):
                        nc.scalar.activation(ru[e], q[2 * e + 1], AF.Square)
                        nc.scalar.activation(tt[e], q[2 * e], AF.Abs)
                        nc.scalar.activation(tt[e], tt[e], AF.Relu,
                                             bias=sc(6 + e))    # overshoot
                        nc.scalar.activation(tt[e], tt[e], AF.Square)
                        nc.vector.tensor_tensor(out=ru[e], in0=ru[e],
                                                in1=tt[e], op=AL.add)
                    nc.vector.tensor_tensor(out=ru[0], in0=ru[0], in1=ru[1],
                                            op=AL.min)
                    nc.vector.tensor_tensor(out=d2_st[:, j, :], in0=ru[0],
                                            in1=ru[2], op=AL.min)

                    cw0 = work.tile([128, TP], FP, tag="cw0")
                    nc.scalar.activation(cw0, q[6], AF.Copy)
                    cw1 = work.tile([128, TP], FP, tag="cw1")
                    nc.scalar.activation(cw1, q[7], AF.Copy)
                    cw2 = work.tile([128, TP], FP, tag="cw2")
                    nc.scalar.activation(cw2, q[8], AF.Copy)
                    m1 = work.tile([128, TP], FP, tag="m1")
                    nc.vector.tensor_tensor(out=m1, in0=cw0, in1=cw1,
                                            op=AL.min)
                    nc.vector.tensor_tensor(out=m1, in0=m1, in1=cw2,
                                            op=AL.min)
                    nc.vector.tensor_scalar(out=sb_st[:, j, :],
                                            in0=m1.bitcast(mybir.dt.uint32),
                                            scalar1=0x80000000, scalar2=None,
                                            op0=AL.bitwise_and)

                    wc0, wc1, wc2 = cw0, cw1, cw2
                    nc.vector.tensor_scalar(out=wc0, in0=cw0, scalar1=0.0,
                                            scalar2=1.0, op0=AL.max,
                                            op1=AL.min)
                    nc.vector.tensor_scalar(out=wc1, in0=cw1, scalar1=0.0,
                                            scalar2=1.0, op0=AL.max,
                                            op1=AL.min)
                    nc.vector.tensor_scalar(out=wc2, in0=cw2, scalar1=0.0,
                                            scalar2=1.0, op0=AL.max,
                                            op1=AL.min)
                    s01 = work.tile([128, TP], FP, tag="s01")
                    nc.vector.tensor_tensor(out=s01, in0=wc0, in1=wc1,
                                            op=AL.add)
                    nc.vector.tensor_tensor(out=s01, in0=s01, in1=wc2,
                                            op=AL.add)
                    invs = work.tile([128, TP], FP, tag="invs")
                    nc.vector.reciprocal_approx_fast(out=invs, in_=s01)
                    r1 = work.tile([128, TP], FP, tag="r1")
                    nc.vector.tensor_scalar(out=r1, in0=wc0, scalar1=sc(0),
                                            scalar2=None, op0=AL.mult)
                    nc.vector.scalar_tensor_tensor(out=r1, in0=wc1,
                                                   scalar=sc(1), in1=r1,
                                                   op0=AL.mult, op1=AL.add)
                    nc.vector.scalar_tensor_tensor(out=r1, in0=wc2,
                                                   scalar=sc(2), in1=r1,
                                                   op0=AL.mult, op1=AL.add)
                    nc.vector.tensor_tensor(out=r1, in0=r1, in1=invs,
                                            op=AL.mult)
                    nc.vector.reciprocal_approx_fast(out=s01, in_=r1)  # zp
                    nc.vector.tensor_scalar(out=zn_st[:, j, :], in0=s01,
                                            scalar1=-1.0 / (FAR - NEAR),
                                            scalar2=FAR / (FAR - NEAR),
                                            op0=AL.mult, op1=AL.add)
                    for k, wck in enumerate([wc0, wc1, wc2]):
                        nc.gpsimd.tensor_tensor(out=wn_st[k][:, j, :],
                                                in0=wck, in1=invs,
                                                op=AL.mult)

            # -- phase 1b over the PAIR: one table load per LUT fn serves 2
            for pos in pair:
                zn_st, pv_st, wn_st, d2_st, sb_st, acc = st8[pos]
                for j in range(pattern[pos]):
                    nc.scalar.activation(d2_st[:, j, :], d2_st[:, j, :],
                                         AF.Sqrt, bias=b_sqrt)
            for pos in pair:
                zn_st, pv_st, wn_st, d2_st, sb_st, acc = st8[pos]
                for j in range(pattern[pos]):
                    nc.vector.tensor_tensor(
                        out=d2_st[:, j, :].bitcast(mybir.dt.uint32),
                        in0=d2_st[:, j, :].bitcast(mybir.dt.uint32),
                        in1=sb_st[:, j, :], op=AL.bitwise_or)
            for pos in pair:
                zn_st, pv_st, wn_st, d2_st, sb_st, acc = st8[pos]
                for j in range(pattern[pos]):
                    nc.scalar.activation(pv_st[:, j, :], d2_st[:, j, :],
                                         AF.Sigmoid, scale=1.0 / SIGMA)
            for pos in pair:
                zn_st, pv_st, wn_st, d2_st, sb_st, acc = st8[pos]
                for j in range(pattern[pos]):
                    q1m = work.tile([128, TP], FP, tag="q1m")
                    nc.vector.tensor_scalar(out=q1m, in0=pv_st[:, j, :],
                                            scalar1=-1.0, scalar2=1.0,
                                            op0=AL.mult, op1=AL.add)
                    lq = work.tile([128, TP], FP, tag="lq")
                    nc.scalar.activation(lq, q1m, AF.Ln, bias=b_ln)
                    nc.tensor.matmul(acc[64:65, :], onesf[:, 0:1], lq,
                                     start=(j == 0),
                                     stop=(j == pattern[pos] - 1))

            # -- phase 3 over the pair: exp weights, rgb/dsum accumulation --
            for pos in pair:
                zn_st, pv_st, wn_st, d2_st, sb_st, acc = st8[pos]
                K = pattern[pos]
                for j in range(K):
                    cj = base[pos] + j
                    d = work.tile([128, TP], FP, tag="d")
                    nc.scalar.activation(d, zn_st[:, j, :], AF.Exp,
                                         scale=1.0 / GAMMA, bias=b_exp)
                    nc.gpsimd.tensor_tensor(out=d, in0=pv_st[:, j, :],
                                            in1=d, op=AL.mult)     # wexp
                    for k in range(3):
                        g = work.tile([128, TP], FP, tag="g", bufs=3)
                        nc.gpsimd.tensor_tensor(out=g, in0=d,
                                                in1=wn_st[k][:, j, :],
                                                op=AL.mult)
                        nc.tensor.matmul(
                            acc[0:3, :],
                            tex_sb[:, cj * 9 + k * 3: cj * 9 + (k + 1) * 3],
                            g, start=(j == 0 and k == 0),
                            stop=(j == K - 1 and k == 2))
                    nc.tensor.matmul(acc[32:33, :], onesf[:, 0:1], d,
                                     start=(j == 0), stop=(j == K - 1))

                o6 = zm.tile([65, TP], FP, tag="o6")
                nc.vector.tensor_copy(o6[0:3, :], acc[0:3, :])
                nc.vector.tensor_copy(o6[32:33, :], acc[32:33, :])
                nc.scalar.activation(o6[64:65, :], acc[64:65, :], AF.Copy)
                nc.sync.dma_start(out=d_out[0:3, pos * TP:(pos + 1) * TP],
                                  in_=o6[0:3, :])
                nc.sync.dma_start(out=d_out[3:4, pos * TP:(pos + 1) * TP],
                                  in_=o6[32:33, :])
                nc.sync.dma_start(out=d_out[4:5, pos * TP:(pos + 1) * TP],
                                  in_=o6[64:65, :])
    nc.compile()
    return nc


def kernel(face_vertices, face_textures):
    prep = _host_prep(face_vertices)
    pattern, assign, mhat = _cull_and_balance(prep)
    totc = sum(pattern)

    pix = ((np.arange(H, dtype=np.float64) + 0.5) / H) * 2.0 - 1.0
    in_maps = []
    for c in range(NCORES):
        coef = np.zeros((totc, NK, 9 * 128), np.float32)
        tex = np.zeros((128, totc * 9), np.float32)
        scal = np.zeros((128, totc * 9), np.float32)
        basis = np.zeros((NPOS, NK, TP), np.float32)
        jj = 0
        for pos in range(NPOS):
            b, faces = assign[c][pos]
            yb, xb = b // 2, b % 2
            py = np.repeat(pix[4 * yb:4 * yb + 4], 128)
            px = np.tile(pix[128 * xb:128 * xb + 128], 4)
            b4 = np.stack([np.ones(TP), px, py, px ** 2 + py ** 2])
            bs = _split3(b4)
            for t, (ci, bi) in enumerate(COMBOS):
                basis[pos, 4 * t:4 * t + 4] = bs[bi].astype(np.float32)
            for j in range(pattern[pos]):
                cf, tx, sl = _face_arrays(prep, face_textures,
                                          faces[j * 128:(j + 1) * 128])
                coef[jj] = cf.reshape(NK, 9 * 128)
                tex[:, jj * 9:(jj + 1) * 9] = tx.reshape(128, 9)
                scal[:, jj * 9:(jj + 1) * 9] = sl
                jj += 1
        in_maps.append({"coef": coef, "basis": basis, "tex": tex, "scal": scal})

    nc = _build_program(pattern, mhat)
    global LAST_RESULT
    if TRACE:
        res = run_bass_kernel_spmd(nc, in_maps, core_ids=list(range(NCORES)),
                                   trace=True)
    else:
        res = run_bass_kernel_spmd(nc, in_maps, core_ids=list(range(NCORES)))
    LAST_RESULT = res

    out = np.zeros((1, 4, H, W), np.float32)
    for c in range(NCORES):
        o6 = res.results[c]["out6"]                        # [6, NPOS*TP]
        for pos in range(NPOS):
            b, _ = assign[c][pos]
            yb, xb = b // 2, b % 2
            seg = o6[:, pos * TP:(pos + 1) * TP]
            wbg = np.float32(np.exp((EPS - mhat) / GAMMA))
            dsum = seg[3] + wbg
            rgb = seg[0:3] / dsum[None]
            alpha = 1.0 - np.exp(seg[4])
            ys = slice(4 * yb, 4 * yb + 4)
            xs = slice(128 * xb, 128 * xb + 128)
            out[0, 0:3, ys, xs] = rgb.reshape(3, 4, 128)
            out[0, 3, ys, xs] = alpha.reshape(4, 128)
    return out



# revision 22
# speedup vs baseline: 1.8320x; 1.8320x over previous
"""SoftRas-style soft rasterizer on 8 Trainium2 NeuronCores — v2.

Per core: 4 image tiles of 16 rows x 128 px (TP=2048). Tiles are dealt to
cores by kept-face count so each tile needs one 128-face chunk.

Math per (face, pixel), validated against the reference in sim4.py:
- PE (bf16 3-way-split coeffs x 6 combos, basis [1,px,py,px2,py2,pxpy],
  K=36): LD2_e (squared edge-line distance, quadratic), U'_e (along-edge
  coord minus half-length, affine), w0/w1/w2 (barycentric, affine,
  per-face scaled by beta for fp16 range).
- d2_e = LD2_e + relu(|U'_e|-h_e)^2: abs on ScalarE (PSUM drain), y/o2 on
  VectorE (bf16), o2 added into the LD2 PSUM bank by an identity matmul.
- d2 = min_e d2_e (VectorE, PSUM); dist = sqrt (ScalarE, bias 1e-6).
- sign: inside <=> sum relu(w) == beta (pre-clamp sum); sign bit of
  (beta*(1+margin) - s01r) ORed into dist -> sd.
- zp: wc = min(relu(w), beta) (= clip(w,0,1) scaled), r1 = sum wc*iz
  (fp16), zp = s01*recip(r1); exp weight e2 = Exp(-C*zp - ln s01 + A)
  folds the zn affine, the softmax shift, and the 1/s01 normalizer.
- p = Sigmoid(sd/SIGMA) bf16 (keeps e^-87 tails: empty pixels get colors
  from the softmax ratio like the reference); lq = Ln(-p+1) = ln(1-p).
- rgb: g_k = wexp*wc_k (bf16) contracted against per-k [128,4] stationary
  (3 tex channels + ones column -> dsum = sum p*e2 exactly).
- Host: rgb/(dsum+wbg), alpha = 1-exp(lnacc).
- Cull: per-tile winner-score map — keep faces that can come within 14
  sigma-units of the guaranteed softmax winner anywhere in the tile
  (empty-pixel colors are decided by tiny-weight RATIOS, so distant
  winners must be kept), plus coverage/alpha rules.
- 3 activation table loads: sqrt set, sigmoid set, natural_log_exp set.
"""
import sys
sys.path.insert(0, '/opt/trn_rl_repo')
import numpy as np
import ml_dtypes
from contextlib import ExitStack

import concourse.bass as bass
import concourse.bacc as bacc
import concourse.tile as tile
import concourse.mybir as mybir
from concourse.bass_utils import run_bass_kernel_spmd
from concourse.masks import make_identity

TRACE = False
LAST_RESULT = None

F_TOT = 512
H = W = 256
NCORES = 8
ROWS = 16
TP = ROWS * 128                # 2048 px per tile
NPOS = (H * W) // (NCORES * TP)  # 4
FD = 512
SIGMA = 1e-2
GAMMA = 1e-3
EPS = 1e-3
NEAR, FAR = 1.0, 100.0
CEXP = 1.0 / ((FAR - NEAR) * GAMMA)
MARGIN = 0.008
FP = mybir.dt.float32
F16 = mybir.dt.float16
BF = mybir.dt.bfloat16
U16 = mybir.dt.uint16
AL = mybir.AluOpType
AF = mybir.ActivationFunctionType

COMBOS = [(0, 0), (0, 1), (1, 0), (0, 2), (1, 1), (2, 0)]
NB = 6                          # basis monomials 1,px,py,px2,py2,pxpy
K = NB * len(COMBOS)            # 36


def _split3(a):
    a = np.asarray(a, np.float64)
    h = a.astype(ml_dtypes.bfloat16).astype(np.float64)
    r = a - h
    m = r.astype(ml_dtypes.bfloat16).astype(np.float64)
    l = (r - m).astype(ml_dtypes.bfloat16).astype(np.float64)
    return [h, m, l]


def _host_prep(face_vertices, face_textures):
    fv = np.asarray(face_vertices, np.float64)[0]
    tex = np.asarray(face_textures, np.float64)[0]
    F = fv.shape[0]
    x = fv[:, :, 0]; y = fv[:, :, 1]; z = fv[:, :, 2]
    x0, x1, x2 = x[:, 0], x[:, 1], x[:, 2]
    y0, y1, y2 = y[:, 0], y[:, 1], y[:, 2]

    den = (y1 - y2) * (x0 - x2) + (x2 - x1) * (y0 - y2)
    den = np.where(np.abs(den) < 1e-10, 1e-10, den)

    LD2c = np.zeros((3, F, 6))
    Uc = np.zeros((3, F, 3))
    HL = np.zeros((3, F))
    for e, (ia, ib) in enumerate([(0, 1), (1, 2), (2, 0)]):
        ax, ay = x[:, ia], y[:, ia]
        dx, dy = x[:, ib] - ax, y[:, ib] - ay
        L = np.sqrt(np.maximum(dx * dx + dy * dy, 1e-12))
        c0 = (ax * dy - ay * dx) / L; cx = -dy / L; cy = dx / L
        LD2c[e, :, 0] = c0 * c0
        LD2c[e, :, 1] = 2 * c0 * cx
        LD2c[e, :, 2] = 2 * c0 * cy
        LD2c[e, :, 3] = cx * cx
        LD2c[e, :, 4] = cy * cy
        LD2c[e, :, 5] = 2 * cx * cy
        Uc[e, :, 0] = (-ax * dx - ay * dy) / L - L / 2
        Uc[e, :, 1] = dx / L
        Uc[e, :, 2] = dy / L
        HL[e] = L / 2

    W0c = np.stack([(-(y1 - y2) * x2 - (x2 - x1) * y2) / den,
                    (y1 - y2) / den, (x2 - x1) / den], -1)
    W1c = np.stack([(-(y2 - y0) * x2 - (x0 - x2) * y2) / den,
                    (y2 - y0) / den, (x0 - x2) / den], -1)
    W2c = -W0c - W1c
    W2c[:, 0] += 1.0
    beta = np.minimum(1.0, 1000.0 / np.maximum.reduce(
        [np.abs(W0c).sum(1), np.abs(W1c).sum(1), np.abs(W2c).sum(1)]))
    Ws = np.stack([W0c * beta[:, None], W1c * beta[:, None],
                   W2c * beta[:, None]])

    iz = 1.0 / z
    assert z.min() > NEAR + 0.05 and z.max() < FAR - 0.05
    znUB = (FAR - z.min(1)) / (FAR - NEAR)
    znLB = (FAR - z.max(1)) / (FAR - NEAR)
    mhat = float(max(znUB.max(), EPS))
    D = znLB.max()
    assert mhat - D <= 0.2
    return dict(LD2c=LD2c, Uc=Uc, HL=HL, Ws=Ws, tex=tex, iz=iz, beta=beta,
                mhat=mhat, D=D, znUB=znUB, znLB=znLB,
                xv=x, yv=y,
                xmin=x.min(1), xmax=x.max(1), ymin=y.min(1), ymax=y.max(1))


def _cull_and_balance(prep):
    pixc = ((np.arange(H) + 0.5) / H) * 2.0 - 1.0
    x, y = prep['xv'], prep['yv']
    # winner-score map on a 64x64 subgrid
    sub = pixc[::4]
    spx = sub[None, None, :]; spy = sub[None, :, None]
    sd2 = None
    for ia, ib in [(0, 1), (1, 2), (2, 0)]:
        ax, ay = x[:, ia, None, None], y[:, ia, None, None]
        dx, dy = x[:, ib, None, None] - ax, y[:, ib, None, None] - ay
        l2 = dx * dx + dy * dy
        t = np.clip(((spx - ax) * dx + (spy - ay) * dy)
                    / np.maximum(l2, 1e-12), 0, 1)
        dd = (spx - (ax + t * dx)) ** 2 + (spy - (ay + t * dy)) ** 2
        sd2 = dd if sd2 is None else np.minimum(sd2, dd)
    score = (-np.sqrt(sd2) / SIGMA
             + ((prep['znLB'] - prep['mhat']) / GAMMA)[:, None, None]).max(0)

    nyb = H // ROWS
    tiles = []
    for yb in range(nyb):
        for xb in range(2):
            tiles.append((pixc[ROWS * yb], pixc[ROWS * yb + ROWS - 1],
                          pixc[128 * xb], pixc[128 * xb + 127]))
    tiles = np.array(tiles)
    ygap = np.maximum(0.0, np.maximum(prep['ymin'][None] - tiles[:, 1:2],
                                      tiles[:, 0:1] - prep['ymax'][None]))
    xgap = np.maximum(0.0, np.maximum(prep['xmin'][None] - tiles[:, 3:4],
                                      tiles[:, 2:3] - prep['xmax'][None]))
    gap = np.sqrt(xgap ** 2 + ygap ** 2)
    smin = np.zeros(len(tiles))
    for b in range(len(tiles)):
        yb, xb = b // 2, b % 2
        smin[b] = score[4 * yb:4 * yb + 4, 32 * xb:32 * xb + 32].min() \
            - 0.034 / SIGMA
    znUB, D = prep['znUB'], prep['D']
    keep = ((gap < 0.12) | ((gap / SIGMA) + (D - znUB) / GAMMA < 25.0) |
            ((-gap / SIGMA + (znUB - prep['mhat']) / GAMMA)
             >= smin[:, None] - 14.0))
    counts = keep.sum(1)
    assert counts.max() <= 128, f"tile face count {counts.max()} > 128"
    order = np.argsort(-counts, kind='stable')
    assign = [[None] * NPOS for _ in range(NCORES)]
    for p in range(NPOS):
        for c in range(NCORES):
            b = int(order[p * NCORES + c])
            faces = np.nonzero(keep[b])[0]
            pad = 128 - len(faces)
            faces = np.concatenate([faces, -np.ones(pad, np.int64)])
            assign[c][p] = (b, faces)
    return assign


def _chunk_arrays(prep, faces):
    """coef [K, 9*128] bf16 (LD2x3, Ux3, Wx3), tex4 [128,12] bf16,
    scal [128, 8] fp32 (iz0-2, hl0-2, bm, bt)."""
    f = np.asarray(faces)
    dummy = f < 0
    fi = np.where(dummy, 0, f)

    co = np.zeros((9, 6, 128))
    for e in range(3):
        co[e, :, :] = prep['LD2c'][e][fi].T
        co[3 + e, 0:3, :] = prep['Uc'][e][fi].T
        co[6 + e, 0:3, :] = prep['Ws'][e][fi].T
    co[:, :, dummy] = 0.0
    co[6, 0, dummy] = -1.0
    co[7, 0, dummy] = -1.0
    co[8, 0, dummy] = 3.0
    cs = _split3(co)
    coefk = np.zeros((K, 9 * 128), ml_dtypes.bfloat16)
    for t, (ci, bi) in enumerate(COMBOS):
        blk = cs[ci]
        for q in range(9):
            coefk[NB * t:NB * t + 6, q * 128:(q + 1) * 128] = \
                blk[q].astype(ml_dtypes.bfloat16)

    tex4 = np.zeros((128, 12), ml_dtypes.bfloat16)
    txf = prep['tex'][fi].copy()
    txf[dummy] = 0.0
    for k in range(3):
        tex4[:, 4 * k:4 * k + 3] = txf[:, k].astype(ml_dtypes.bfloat16)
        tex4[:, 4 * k + 3] = np.where(dummy, 0.0, 1.0)

    scal = np.zeros((128, 8), np.float32)
    izf = prep['iz'][fi].copy()
    izf[dummy] = 0.011
    scal[:, 0:3] = izf
    hlf = prep['HL'][:, fi].T.copy()
    hlf[dummy] = -10.0
    scal[:, 3:6] = hlf
    bt = prep['beta'][fi].copy()
    bt[dummy] = 1.0
    scal[:, 6] = bt * (1 + MARGIN)
    scal[:, 7] = bt
    return coefk, tex4, scal


def _build_program(mhat):
    nc = bacc.Bacc("TRN2", target_bir_lowering=False, debug=False,
                   num_devices=NCORES)
    d_coef = nc.dram_tensor("coef", [NPOS, K, 9 * 128], BF,
                            kind="ExternalInput")
    d_basis = nc.dram_tensor("basis", [NPOS, K, TP], BF, kind="ExternalInput")
    d_tex = nc.dram_tensor("tex4", [128, NPOS * 12], BF, kind="ExternalInput")
    d_scal = nc.dram_tensor("scal", [128, NPOS * 8], FP, kind="ExternalInput")
    d_out = nc.dram_tensor("out", [5, NPOS * TP], FP, kind="ExternalOutput")

    Abias = (FAR / (FAR - NEAR) - mhat) / GAMMA

    with ExitStack() as ctx:
        tc = ctx.enter_context(tile.TileContext(nc))
        const = ctx.enter_context(tc.tile_pool(name="const", bufs=1))
        coefp = ctx.enter_context(tc.tile_pool(name="coefp", bufs=2))
        basp = ctx.enter_context(tc.tile_pool(name="basp", bufs=2))
        store = ctx.enter_context(tc.tile_pool(name="store", bufs=4))
        work = ctx.enter_context(tc.tile_pool(name="work", bufs=2))
        qp = ctx.enter_context(tc.tile_pool(name="qp", bufs=3, space="PSUM"))
        accp = ctx.enter_context(tc.tile_pool(name="accp", bufs=2,
                                              space="PSUM"))

        onesb = const.tile([128, 1], BF)
        nc.vector.memset(onesb, 1.0)
        b_exp = const.tile([128, 1], FP)
        nc.vector.memset(b_exp, Abias)
        b_sqrt = const.tile([128, 1], FP)
        nc.vector.memset(b_sqrt, 1e-6)
        ident = const.tile([128, 128], BF)
        make_identity(nc, ident)
        tex_sb = const.tile([128, NPOS * 12], BF)
        nc.sync.dma_start(out=tex_sb, in_=d_tex[:, :])
        scal_sb = const.tile([128, NPOS * 8], FP)
        nc.sync.dma_start(out=scal_sb, in_=d_scal[:, :])

        sd_t = [None] * NPOS; p_t = [None] * NPOS; zp_t = [None] * NPOS
        s01_t = [None] * NPOS; sb_t = [None] * NPOS
        wc_t = [None] * NPOS; lq_t = [None] * NPOS; e2_t = [None] * NPOS

        def mm2(qt, coef, q, bas, half):
            """two FD512 matmuls filling qt [128,1024] from quantity q."""
            for s in range(2):
                nc.tensor.matmul(qt[:, s * FD:(s + 1) * FD],
                                 coef[:, q * 128:(q + 1) * 128],
                                 bas[:, half * 1024 + s * FD:
                                     half * 1024 + (s + 1) * FD],
                                 start=True, stop=True)

        # ================= phase 1a (no LUT fns beyond abs/relu) ==========
        for pos in range(NPOS):
            coef = coefp.tile([K, 9 * 128], BF, tag="coef")
            nc.sync.dma_start(out=coef, in_=d_coef[pos, :, :])
            bas = basp.tile([K, TP], BF, tag="bas")
            nc.sync.dma_start(out=bas, in_=d_basis[pos, :, :])
            sc = lambda i: scal_sb[:, pos * 8 + i:pos * 8 + i + 1]

            # |U'| for 3 edges (S drains PSUM -> bf16)
            aU = [work.tile([128, TP], BF, tag=f"aU{e}", name=f"aU{e}")
                  for e in range(3)]
            for half in range(2):
                for e in range(3):
                    qu = qp.tile([128, 1024], FP, tag="q", name=f"qu{e}")
                    mm2(qu, coef, 3 + e, bas, half)
                    nc.scalar.activation(
                        aU[e][:, half * 1024:(half + 1) * 1024], qu, AF.Abs)
            # o2_e = relu(|U'|-h)^2 (V, bf16, in-place chain)
            for e in range(3):
                nc.vector.tensor_scalar(out=aU[e], in0=aU[e],
                                        scalar1=sc(3 + e), scalar2=None,
                                        op0=AL.subtract)
                nc.vector.scalar_tensor_tensor(out=aU[e], in0=aU[e],
                                               scalar=0.0, in1=aU[e],
                                               op0=AL.max, op1=AL.mult)
            # d2 = min_e (LD2_e + o2_e); sqrt later (table batching)
            dsd = store.tile([128, TP], F16, tag="dsd", name="dsd")
            sd_t[pos] = dsd
            for half in range(2):
                m1 = work.tile([128, 1024], FP, tag="m1")
                for e in range(3):
                    ql = qp.tile([128, 1024], FP, tag="q", name=f"ql{e}")
                    for s in range(2):
                        nc.tensor.matmul(
                            ql[:, s * FD:(s + 1) * FD],
                            coef[:, e * 128:(e + 1) * 128],
                            bas[:, half * 1024 + s * FD:
                                half * 1024 + (s + 1) * FD],
                            start=True, stop=False)
                        nc.tensor.matmul(
                            ql[:, s * FD:(s + 1) * FD], ident,
                            aU[e][:, half * 1024 + s * FD:
                                  half * 1024 + (s + 1) * FD],
                            start=False, stop=True)
                    if e == 0:
                        nc.scalar.activation(m1, ql, AF.Copy)
                    elif e == 1:
                        nc.vector.tensor_tensor(out=m1, in0=m1, in1=ql,
                                                op=AL.min)
                    else:
                        nc.vector.tensor_tensor(
                            out=dsd[:, half * 1024:(half + 1) * 1024],
                            in0=m1, in1=ql, op=AL.min)

            # W drains -> wr (relu, fp16)
            wc = [store.tile([128, TP], F16, tag=f"wc{k}", name=f"wc{k}")
                  for k in range(3)]
            wc_t[pos] = wc
            for half in range(2):
                for k in range(3):
                    qw = qp.tile([128, 1024], FP, tag="q", name=f"qw{k}")
                    mm2(qw, coef, 6 + k, bas, half)
                    nc.scalar.activation(
                        wc[k][:, half * 1024:(half + 1) * 1024], qw, AF.Relu)
            # sign from pre-clamp sum (u1 reused in place for the sum)
            u1 = work.tile([128, TP], F16, tag="u1", name="s01r")
            nc.vector.tensor_tensor(out=u1, in0=wc[0], in1=wc[1], op=AL.add)
            nc.vector.tensor_tensor(out=u1, in0=u1, in1=wc[2], op=AL.add)
            vv = work.tile([128, TP], F16, tag="vv")
            nc.vector.tensor_scalar(out=vv, in0=u1, scalar1=-1.0,
                                    scalar2=sc(6), op0=AL.mult, op1=AL.add)
            sb = store.tile([128, TP], U16, tag="sb", name="sb")
            sb_t[pos] = sb
            nc.vector.tensor_scalar(out=sb, in0=vv.bitcast(U16),
                                    scalar1=0x8000, scalar2=None,
                                    op0=AL.bitwise_and)
            # clamp wc to beta (= clip(w,0,1) scaled), in place
            for k in range(3):
                nc.vector.tensor_scalar(out=wc[k], in0=wc[k], scalar1=sc(7),
                                        scalar2=None, op0=AL.min)
            u2 = work.tile([128, TP], F16, tag="u1", name="s01c")
            nc.vector.tensor_tensor(out=u2, in0=wc[0], in1=wc[1], op=AL.add)
            s01 = store.tile([128, TP], F16, tag="s01", name="s01")
            s01_t[pos] = s01
            nc.vector.tensor_tensor(out=s01, in0=u2, in1=wc[2], op=AL.add)
            # r1 (fp16 products, fp32 for reciprocal; rr in place)
            p1 = work.tile([128, TP], F16, tag="p1")
            nc.vector.tensor_scalar(out=p1, in0=wc[1], scalar1=sc(1),
                                    scalar2=None, op0=AL.mult)
            p2 = work.tile([128, TP], F16, tag="vv", name="p2")
            nc.vector.tensor_scalar(out=p2, in0=wc[2], scalar1=sc(2),
                                    scalar2=None, op0=AL.mult)
            nc.vector.tensor_tensor(out=p1, in0=p1, in1=p2, op=AL.add)
            r1 = work.tile([128, TP], FP, tag="r1")
            nc.vector.scalar_tensor_tensor(out=r1, in0=wc[0], scalar=sc(0),
                                           in1=p1, op0=AL.mult, op1=AL.add)
            nc.vector.reciprocal_approx_fast(out=r1, in_=r1)
            zp = store.tile([128, TP], F16, tag="zp", name="zp")
            zp_t[pos] = zp
            nc.vector.tensor_tensor(out=zp, in0=s01, in1=r1, op=AL.mult)

        # ================= sqrt set =======================================
        for pos in range(NPOS):
            nc.scalar.activation(sd_t[pos], sd_t[pos], AF.Sqrt, bias=b_sqrt)
        for pos in range(NPOS):
            nc.vector.tensor_tensor(out=sd_t[pos].bitcast(U16),
                                    in0=sd_t[pos].bitcast(U16),
                                    in1=sb_t[pos], op=AL.bitwise_or)
        # ================= sigmoid set (p overwrites sd in place) =========
        for pos in range(NPOS):
            pt = sd_t[pos].bitcast(BF)
            p_t[pos] = pt
            nc.scalar.activation(pt, sd_t[pos], AF.Sigmoid, scale=1.0 / SIGMA)
        # ================= ln/exp set =====================================
        for pos in range(NPOS):
            lq = sb_t[pos].bitcast(BF)          # sb dead after or-sign
            lq_t[pos] = lq
            nc.scalar.activation(lq, p_t[pos], AF.Ln, scale=-1.0, bias=1.0)
        for pos in range(NPOS):
            lns = s01_t[pos]                    # lns overwrites s01 in place
            nc.scalar.activation(lns, s01_t[pos], AF.Ln)
            xarg = zp_t[pos]                    # xarg overwrites zp in place
            nc.vector.scalar_tensor_tensor(out=xarg, in0=zp_t[pos],
                                           scalar=-CEXP, in1=lns,
                                           op0=AL.mult, op1=AL.subtract)
            e2 = zp_t[pos].bitcast(BF)          # e2 overwrites xarg in place
            e2_t[pos] = e2
            nc.scalar.activation(e2, xarg, AF.Exp, bias=b_exp)
        # ================= accumulate + out ===============================
        for pos in range(NPOS):
            wexp = p_t[pos]                     # wexp = p*e2 in place over p
            nc.vector.tensor_tensor(out=wexp, in0=p_t[pos], in1=e2_t[pos],
                                    op=AL.mult)
            g = [wc_t[pos][k].bitcast(BF) for k in range(3)]
            for k in range(3):
                nc.vector.tensor_tensor(out=g[k], in0=wexp, in1=wc_t[pos][k],
                                        op=AL.mult)
            for s in range(4):
                acc = accp.tile([33, FD], FP, tag="acc")
                for k in range(3):
                    nc.tensor.matmul(
                        acc[0:4, :],
                        tex_sb[:, pos * 12 + 4 * k:pos * 12 + 4 * k + 4],
                        g[k][:, s * FD:(s + 1) * FD],
                        start=(k == 0), stop=(k == 2))
                nc.tensor.matmul(acc[32:33, :], onesb[:, 0:1],
                                 lq_t[pos][:, s * FD:(s + 1) * FD],
                                 start=True, stop=True)
                o5 = work.tile([33, FD], FP, tag="o5", name="o5")
                nc.scalar.activation(o5, acc, AF.Copy)
                nc.sync.dma_start(
                    out=d_out[0:4, pos * TP + s * FD:pos * TP + (s + 1) * FD],
                    in_=o5[0:4, :])
                nc.sync.dma_start(
                    out=d_out[4:5, pos * TP + s * FD:pos * TP + (s + 1) * FD],
                    in_=o5[32:33, :])
    nc.compile()
    return nc


def kernel(face_vertices, face_textures):
    prep = _host_prep(face_vertices, face_textures)
    assign = _cull_and_balance(prep)

    pix = ((np.arange(H, dtype=np.float64) + 0.5) / H) * 2.0 - 1.0
    in_maps = []
    for c in range(NCORES):
        coef = np.zeros((NPOS, K, 9 * 128), ml_dtypes.bfloat16)
        basis = np.zeros((NPOS, K, TP), ml_dtypes.bfloat16)
        tex4 = np.zeros((128, NPOS * 12), ml_dtypes.bfloat16)
        scal = np.zeros((128, NPOS * 8), np.float32)
        for pos in range(NPOS):
            b, faces = assign[c][pos]
            yb, xb = b // 2, b % 2
            pyv = np.repeat(pix[ROWS * yb:ROWS * yb + ROWS], 128)
            pxv = np.tile(pix[128 * xb:128 * xb + 128], ROWS)
            b6 = np.stack([np.ones(TP), pxv, pyv, pxv * pxv, pyv * pyv,
                           pxv * pyv])
            bsplit = _split3(b6)
            for t, (ci, bi) in enumerate(COMBOS):
                basis[pos, NB * t:NB * t + 6] = \
                    bsplit[bi].astype(ml_dtypes.bfloat16)
            cf, tx, sl = _chunk_arrays(prep, faces)
            coef[pos] = cf
            tex4[:, pos * 12:(pos + 1) * 12] = tx
            scal[:, pos * 8:(pos + 1) * 8] = sl
        in_maps.append({"coef": coef, "basis": basis, "tex4": tex4,
                        "scal": scal})

    nc = _build_program(prep['mhat'])
    global LAST_RESULT
    res = run_bass_kernel_spmd(nc, in_maps, core_ids=list(range(NCORES)),
                               trace=TRACE)
    LAST_RESULT = res

    wbg = np.float64(np.exp((EPS - prep['mhat']) / GAMMA))
    out = np.zeros((1, 4, H, W), np.float32)
    for c in range(NCORES):
        o = res.results[c]["out"]                          # [5, NPOS*TP]
        for pos in range(NPOS):
            b, _ = assign[c][pos]
            yb, xb = b // 2, b % 2
            seg = o[:, pos * TP:(pos + 1) * TP].astype(np.float64)
            dsum = seg[3] + wbg
            rgb = seg[0:3] / np.where(dsum <= 0, 1.0, dsum)[None]
            alpha = 1.0 - np.exp(seg[4])
            ys = slice(ROWS * yb, ROWS * yb + ROWS)
            xs = slice(128 * xb, 128 * xb + 128)
            out[0, 0:3, ys, xs] = rgb.reshape(3, ROWS, 128)
            out[0, 3, ys, xs] = alpha.reshape(ROWS, 128)
    return out
